# revision 65
# baseline (speedup 1.0000x reference)
"""RNN-T joint network kernel for 8 Trainium2 NeuronCores.

logits[b,t,u,v] = tanh(enc_out[b,t]@W_enc + b_enc + pred_out[b,u]@W_pred + b_pred) @ W_joint + b_joint

Sharding: T axis split 8 ways (32 t's per core). Each core computes its
(B=4, T/8=32, U=64, V=1024) logit slab independently; no collectives.

Per-core dataflow (activations kept transposed, J on partitions, bf16
compute with f32 PSUM accumulate and f32 output):
  encT/predT   : transposed on the HOST in make_in_maps, DMA'd directly
                 in [E-part, row] layout (no on-device transposes)
  PE warm-up   : ~2us of discarded matmuls at kernel start so the HAM
                 clock ramp is at full rate when projections begin
  enc/pred proj: PE bf16 matmuls (W natural layout as lhsT), interleaved
                 per-jc with the first slab's add+tanh for early start
  joint        : DVE broadcast-add (0-step APs) + ACT tanh -> bf16 slab,
                 produced one group ahead of its output tiles
  logits       : PE bf16 matmuls, w_joint streaming as moving operand,
                 out tile [128 btu, 1024 v]; PSUM-evac + b_joint (bf16) add
                 on DVE for vh0/even halves, ACT+GPSIMD for odd-tile vh1
                 (all-DVE for the last 8 tiles so the tail drains fast)
  store        : HWDGE DMA, 256KB per half-tile right after its evac;
                 first two groups run all vh0 halves before vh1 to hide
                 the second w_joint half's DMA arrival
"""

import sys

for _p in ("/opt/trn_rl_repo", "/root/.axon_site/_ro/trn_rl_repo"):
    if _p not in sys.path:
        sys.path.insert(0, _p)

import numpy as np

import concourse.tile as tile
from concourse import bacc, mybir
from concourse import bass_utils
from concourse.ap import AP

F32 = mybir.dt.float32
BF16 = mybir.dt.bfloat16
NP_BF16 = mybir.dt.np(BF16)
TANH = mybir.ActivationFunctionType.Tanh
ADD = mybir.AluOpType.add

N_CORES = 8
B, T, U = 4, 256, 64
TL = T // N_CORES          # 32 t's per core
E = P = J = 512
V = 1024
JC = J // 128              # 4 j-chunks
BT = B * TL                # 128 (b,t) rows per core
NT = BT * U // 128         # 64 output tiles of 128 btu rows
TG = 4                     # t-groups per b (8 t's each -> slab of 512 btu)

_cache = {}


def _build():
    nc = bacc.Bacc("TRN2", target_bir_lowering=False, debug=False,
                   num_devices=N_CORES)

    enc_d = nc.dram_tensor("enc", [128, JC * BT], BF16, kind="ExternalInput").ap()
    pred_d = nc.dram_tensor("pred", [128, JC * B * U], BF16, kind="ExternalInput").ap()
    w_enc_d = nc.dram_tensor("w_enc", [E, J], BF16, kind="ExternalInput").ap()
    w_pred_d = nc.dram_tensor("w_pred", [P, J], BF16, kind="ExternalInput").ap()
    w_joint_d = nc.dram_tensor("w_joint", [J, V], BF16, kind="ExternalInput").ap()
    bias_ep_d = nc.dram_tensor("bias_ep", [128, JC], F32, kind="ExternalInput").ap()
    bias_j_d = nc.dram_tensor("bias_j", [128, V], BF16, kind="ExternalInput").ap()
    out_d = nc.dram_tensor("out", [BT * U, V], F32, kind="ExternalOutput").ap()

    with tile.TileContext(nc) as tc:
        with (
            tc.tile_pool(name="const", bufs=1) as cp,
            tc.tile_pool(name="psum_mm", bufs=6, space="PSUM") as ps_mm,
            tc.tile_pool(name="slab", bufs=4) as slab_pool,
            tc.tile_pool(name="outp", bufs=8) as out_pool,
        ):
            # ---- constant loads, ordered by when the pipeline needs them ----
            encT = cp.tile([128, JC, BT], BF16)
            nc.sync.dma_start(
                encT[:], enc_d.rearrange("p (c b) -> p c b", b=BT))
            w_enc_sb = cp.tile([128, JC, J], BF16)
            we_re = w_enc_d.rearrange("(c p) j -> p c j", p=128)
            nc.sync.dma_start(w_enc_sb[:], we_re[:])
            predT = cp.tile([128, JC, B * U], BF16)
            nc.sync.dma_start(
                predT[:], pred_d.rearrange("p (c b) -> p c b", b=B * U))
            w_pred_sb = cp.tile([128, JC, J], BF16)
            wp_re = w_pred_d.rearrange("(c p) j -> p c j", p=128)
            nc.sync.dma_start(w_pred_sb[:], wp_re[:])
            bias_ep_sb = cp.tile([128, JC], F32)
            nc.sync.dma_start(bias_ep_sb[:], bias_ep_d[:])
            w_joint_sb = cp.tile([128, JC, V], BF16)
            bias_j_sb = cp.tile([128, V], BF16)
            wj_re = w_joint_d.rearrange("(c p) v -> p c v", p=128)
            nc.sync.dma_start(w_joint_sb[:, :, 0:512], wj_re[:, :, 0:512])
            nc.sync.dma_start(bias_j_sb[:], bias_j_d[:])
            nc.sync.dma_start(w_joint_sb[:, :, 512:1024], wj_re[:, :, 512:1024])
            # ---- PE warm-up: ~2us of discarded matmuls so the clock
            # ramp (HAM) is at full rate when the projections start ----
            wtile = cp.tile([128, 512], BF16)
            nc.gpsimd.memset(wtile[:], 0.0)
            wpsum = ps_mm.tile([128, 512], F32, tag="mm")
            for i in range(10):
                nc.tensor.matmul(wpsum[:], wtile[:, 0:128], wtile[:],
                                 start=(i == 0), stop=(i == 9))
            wdump = cp.tile([128, 16], F32)
            nc.scalar.copy(wdump[:], wpsum[:, 0:16])

            # ---- projections (transposed layout [j, row]), interleaved
            # per-jc so the first slab chunk is ready ASAP ----
            encP = cp.tile([128, JC, BT], BF16)
            predP = cp.tile([128, JC, B * U], BF16)
            slab0 = slab_pool.tile([128, JC, 512], BF16)

            def slab_add_tanh(slab, b, tg, jc):
                bt0 = b * TL + tg * 8
                # [128, 8t, 64u] = pred[., u] (bcast t) + enc[., t] (bcast u)
                p_ap = predP[:, jc, b * U:(b + 1) * U]
                in0 = AP(p_ap.tensor, p_ap.offset,
                         [p_ap.ap[0], [0, 8], [1, U]])
                e_ap = encP[:, jc, bt0:bt0 + 8]
                in1 = AP(e_ap.tensor, e_ap.offset,
                         [e_ap.ap[0], [1, 8], [0, U]])
                dst = slab[:, jc, :].rearrange("p (t u) -> p t u", t=8)
                nc.vector.tensor_tensor(dst, in0, in1, ADD)
                nc.scalar.activation(slab[:, jc, :], slab[:, jc, :], TANH)

            for jc in range(JC):
                pe = ps_mm.tile([128, 512], F32, tag="mm")
                for ec in range(JC):
                    nc.tensor.matmul(pe[:, 0:BT],
                                     w_enc_sb[:, ec, jc * 128:(jc + 1) * 128],
                                     encT[:, ec, :],
                                     start=(ec == 0), stop=(ec == JC - 1))
                nc.scalar.copy(encP[:, jc, :], pe[:, 0:BT])
                pp = ps_mm.tile([128, 512], F32, tag="mm")
                for ec in range(JC):
                    nc.tensor.matmul(pp[:, 0:256],
                                     w_pred_sb[:, ec, jc * 128:(jc + 1) * 128],
                                     predT[:, ec, :],
                                     start=(ec == 0), stop=(ec == JC - 1))
                # fused (b_enc + b_pred) bias add during PSUM evacuation
                nc.vector.tensor_scalar_add(predP[:, jc, :], pp[:, 0:256],
                                            bias_ep_sb[:, jc:jc + 1])
                # first slab chunk rides right behind its projections
                slab_add_tanh(slab0, 0, 0, jc)

            # ---- main loop: per (b, t-group of 8) slab -> 4 out tiles.
            # Slabs are produced one group ahead of their tiles so DVE/ACT
            # slab work never gates the PE matmul stream. ----
            slabs = {(0, 0): slab0}

            def make_slab(b, tg):
                s = slab_pool.tile([128, JC, 512], BF16, name="slab",
                                   tag="slab")
                for jc in range(JC):
                    slab_add_tanh(s, b, tg, jc)
                slabs[(b, tg)] = s

            groups = [(b, tg) for b in range(B) for tg in range(TG)]
            for gi, (b, tg) in enumerate(groups):
                    if gi + 1 < len(groups):
                        make_slab(*groups[gi + 1])
                    slab = slabs.pop((b, tg))

                    ots = {}

                    def half_tile(pi, vh):
                        tidx = (b * TG + tg) * 4 + pi
                        if pi not in ots:
                            ots[pi] = out_pool.tile([128, V], F32,
                                                    name="ot", tag="ot")
                        ot = ots[pi]
                        vs = slice(vh * 512, (vh + 1) * 512)
                        po = ps_mm.tile([128, 512], F32, tag="mm")
                        for jc in range(JC):
                            nc.tensor.matmul(
                                po[:],
                                slab[:, jc, pi * 128:(pi + 1) * 128],
                                w_joint_sb[:, jc, vs],
                                start=(jc == 0), stop=(jc == JC - 1))
                        if tidx % 2 == 0 or tidx >= NT - 8:
                            # DVE: fused PSUM evac + b_joint add
                            nc.vector.tensor_tensor(
                                ot[:, vs], po[:], bias_j_sb[:, vs], ADD)
                        else:
                            # ACT evacuates, GPSIMD adds bias in place
                            nc.scalar.copy(ot[:, vs], po[:])
                            nc.gpsimd.tensor_add(
                                ot[:, vs], ot[:, vs], bias_j_sb[:, vs])

                    if gi < 2:
                        # vh0 for all 4 tiles first: hides the w_joint vh1
                        # DMA arrival behind the first 3.4us of matmuls
                        order = [(pi, 0) for pi in range(4)] +                                 [(pi, 1) for pi in range(4)]
                    else:
                        order = [(pi, vh) for pi in range(4) for vh in range(2)]
                    for pi, vh in order:
                        half_tile(pi, vh)
                        tidx = (b * TG + tg) * 4 + pi
                        vs = slice(vh * 512, (vh + 1) * 512)
                        nc.sync.dma_start(
                            out_d[tidx * 128:(tidx + 1) * 128, vs],
                            ots[pi][:, vs])
    nc.compile()
    return nc


def _get_nc():
    if "nc" not in _cache:
        _cache["nc"] = _build()
    return _cache["nc"]


def make_in_maps(enc_out, pred_out, W_enc, b_enc, W_pred, b_pred, W_joint, b_joint):
    predT = np.ascontiguousarray(
        np.asarray(pred_out, dtype=np.float32).reshape(B * U, JC, 128)
        .transpose(2, 1, 0).reshape(128, JC * B * U)).astype(NP_BF16)
    bias_ep = np.ascontiguousarray(
        (np.asarray(b_enc, dtype=np.float32)
         + np.asarray(b_pred, dtype=np.float32)).reshape(JC, 128).T)
    bias_j = np.ascontiguousarray(
        np.broadcast_to(np.asarray(b_joint, dtype=np.float32), (128, V))
    ).astype(NP_BF16)
    w_enc = np.ascontiguousarray(W_enc, dtype=np.float32).astype(NP_BF16)
    w_pred = np.ascontiguousarray(W_pred, dtype=np.float32).astype(NP_BF16)
    w_joint = np.ascontiguousarray(W_joint, dtype=np.float32).astype(NP_BF16)
    enc_f32 = np.asarray(enc_out, dtype=np.float32)
    in_maps = []
    for i in range(N_CORES):
        enc_slab = np.ascontiguousarray(
            enc_f32[:, i * TL:(i + 1) * TL, :].reshape(BT, JC, 128)
            .transpose(2, 1, 0).reshape(128, JC * BT)).astype(NP_BF16)
        in_maps.append({
            "enc": enc_slab, "pred": predT,
            "w_enc": w_enc, "w_pred": w_pred, "w_joint": w_joint,
            "bias_ep": bias_ep, "bias_j": bias_j,
        })
    return in_maps


def assemble(results):
    return np.concatenate(
        [r["out"].reshape(B, TL, U, V) for r in results], axis=1)


def _axon_active():
    try:
        from concourse.bass_utils import axon_active
        return axon_active()
    except Exception:
        return False


def _get_fast_runner(nc):
    """Cached jit dispatch (axon path). Same mechanism as
    bass2jax.run_bass_via_pjrt, built once so repeat kernel() calls skip
    the per-call trace/lower/compile."""
    if "runner" in _cache:
        return _cache["runner"]

    import jax
    from jax.sharding import Mesh, PartitionSpec, NamedSharding
    from jax.experimental.shard_map import shard_map
    from concourse.bass2jax import (
        _bass_exec_p, install_neuronx_cc_hook, partition_id_tensor)

    install_neuronx_cc_hook()
    partition_name = nc.partition_id_tensor.name if nc.partition_id_tensor else None
    in_names, out_names, out_avals, zero_outs = [], [], [], []
    for alloc in nc.m.functions[0].allocations:
        if not isinstance(alloc, mybir.MemoryLocationSet):
            continue
        name = alloc.memorylocations[0].name
        if alloc.kind == "ExternalInput":
            if name != partition_name:
                in_names.append(name)
        elif alloc.kind == "ExternalOutput":
            shape = tuple(alloc.tensor_shape)
            dtype = mybir.dt.np(alloc.dtype)
            out_names.append(name)
            out_avals.append(jax.core.ShapedArray(shape, dtype))
            zero_outs.append(np.zeros(shape, dtype))
    n_params = len(in_names)
    n_outs = len(out_avals)
    all_names = in_names + out_names
    if partition_name is not None:
        all_names = all_names + [partition_name]

    def _body(*args):
        operands = list(args)
        if partition_name is not None:
            operands.append(partition_id_tensor())
        outs = _bass_exec_p.bind(
            *operands, out_avals=tuple(out_avals), in_names=tuple(all_names),
            out_names=tuple(out_names), lowering_input_output_aliases=(),
            sim_require_finite=True, sim_require_nnan=True, nc=nc)
        return tuple(outs)

    devices = jax.devices()[:N_CORES]
    mesh = Mesh(np.asarray(devices), ("core",))
    sharded = jax.jit(
        shard_map(_body, mesh=mesh,
                  in_specs=(PartitionSpec("core"),) * (n_params + n_outs),
                  out_specs=(PartitionSpec("core"),) * n_outs,
                  check_rep=False),
        keep_unused=True)
    sh = NamedSharding(mesh, PartitionSpec("core"))
    zeros_dev = [
        jax.device_put(np.zeros((N_CORES * z.shape[0], *z.shape[1:]), z.dtype), sh)
        for z in zero_outs]

    def run(in_maps):
        concat_in = [
            jax.device_put(
                np.concatenate([in_maps[c][n] for c in range(N_CORES)], axis=0), sh)
            for n in in_names]
        outs = sharded(*concat_in, *zeros_dev)
        res = []
        for c in range(N_CORES):
            m = {}
            for i, n in enumerate(out_names):
                rows = out_avals[i].shape[0]
                m[n] = np.asarray(outs[i][c * rows:(c + 1) * rows])
            res.append(m)
        return res

    _cache["runner"] = run
    return run


def kernel(enc_out, pred_out, W_enc, b_enc, W_pred, b_pred, W_joint, b_joint):
    nc = _get_nc()
    in_maps = make_in_maps(enc_out, pred_out, W_enc, b_enc, W_pred, b_pred,
                           W_joint, b_joint)
    if _axon_active():
        results = _get_fast_runner(nc)(in_maps)
    else:
        results = bass_utils.run_bass_kernel_spmd(
            nc, in_maps, list(range(N_CORES))).results
    return assemble(results)


# revision 71
# speedup vs baseline: 2.3151x; 2.3151x over previous
"""RNN-T joint network kernel for 8 Trainium2 NeuronCores.

logits[b,t,u,v] = tanh(enc_out[b,t]@W_enc + b_enc + pred_out[b,u]@W_pred + b_pred) @ W_joint + b_joint

Sharding: T axis split 8 ways (32 t's per core). Each core computes its
(B=4, T/8=32, U=64, V=1024) logit slab independently; no collectives.

Per-core dataflow (activations kept transposed, J on partitions, bf16
compute with f32 PSUM accumulate and f32 output):
  encT/predT   : transposed on the HOST in make_in_maps, DMA'd directly
                 in [E-part, row] layout (no on-device transposes)
  PE warm-up   : ~2us of discarded matmuls at kernel start so the HAM
                 clock ramp is at full rate when projections begin
  enc/pred proj: PE bf16 matmuls (W natural layout as lhsT), interleaved
                 per-jc with the first slab's add+tanh for early start
  joint        : DVE broadcast-add (0-step APs) + ACT tanh -> bf16 slab,
                 produced one group ahead of its output tiles
  logits       : PE bf16 matmuls, w_joint streaming as moving operand,
                 out tile [128 btu, 1024 v]; PSUM-evac + b_joint (bf16) add
                 on DVE for vh0/even halves, ACT+GPSIMD for odd-tile vh1
                 (all-DVE for the last 8 tiles so the tail drains fast)
  store        : HWDGE DMA, 256KB per half-tile right after its evac;
                 first two groups run all vh0 halves before vh1 to hide
                 the second w_joint half's DMA arrival
"""

import sys

for _p in ("/opt/trn_rl_repo", "/root/.axon_site/_ro/trn_rl_repo"):
    if _p not in sys.path:
        sys.path.insert(0, _p)

import numpy as np

import concourse.tile as tile
from concourse import bacc, mybir
from concourse import bass_utils
from concourse.ap import AP

F32 = mybir.dt.float32
BF16 = mybir.dt.bfloat16
NP_BF16 = mybir.dt.np(BF16)
TANH = mybir.ActivationFunctionType.Tanh
ADD = mybir.AluOpType.add

N_CORES = 8
B, T, U = 4, 256, 64
TL = T // N_CORES          # 32 t's per core
E = P = J = 512
V = 1024
JC = J // 128              # 4 j-chunks
BT = B * TL                # 128 (b,t) rows per core
NT = BT * U // 128         # 64 output tiles of 128 btu rows
TG = 4                     # t-groups per b (8 t's each -> slab of 512 btu)

_cache = {}


def _build():
    nc = bacc.Bacc("TRN2", target_bir_lowering=False, debug=False,
                   num_devices=N_CORES)

    enc_d = nc.dram_tensor("enc", [128, JC * BT], BF16, kind="ExternalInput").ap()
    pred_d = nc.dram_tensor("pred", [128, JC * B * U], BF16, kind="ExternalInput").ap()
    w_enc_d = nc.dram_tensor("w_enc", [E, J], BF16, kind="ExternalInput").ap()
    w_pred_d = nc.dram_tensor("w_pred", [P, J], BF16, kind="ExternalInput").ap()
    w_joint_d = nc.dram_tensor("w_joint", [J, V], BF16, kind="ExternalInput").ap()
    bias_ep_d = nc.dram_tensor("bias_ep", [128, JC], F32, kind="ExternalInput").ap()
    bias_j_d = nc.dram_tensor("bias_j", [128, V], BF16, kind="ExternalInput").ap()
    out_d = nc.dram_tensor("out", [BT * U, V], F32, kind="ExternalOutput").ap()

    with tile.TileContext(nc) as tc:
        with (
            tc.tile_pool(name="const", bufs=1) as cp,
            tc.tile_pool(name="psum_mm", bufs=6, space="PSUM") as ps_mm,
            tc.tile_pool(name="slab", bufs=4) as slab_pool,
            tc.tile_pool(name="outp", bufs=8) as out_pool,
        ):
            # ---- constant loads, ordered by when the pipeline needs them ----
            encT = cp.tile([128, JC, BT], BF16)
            nc.scalar.dma_start(
                encT[:], enc_d.rearrange("p (c b) -> p c b", b=BT))
            w_enc_sb = cp.tile([128, JC, J], BF16)
            we_re = w_enc_d.rearrange("(c p) j -> p c j", p=128)
            nc.sync.dma_start(w_enc_sb[:], we_re[:])
            predT = cp.tile([128, JC, B * U], BF16)
            nc.scalar.dma_start(
                predT[:], pred_d.rearrange("p (c b) -> p c b", b=B * U))
            w_pred_sb = cp.tile([128, JC, J], BF16)
            wp_re = w_pred_d.rearrange("(c p) j -> p c j", p=128)
            nc.sync.dma_start(w_pred_sb[:], wp_re[:])
            bias_ep_sb = cp.tile([128, JC], F32)
            nc.sync.dma_start(bias_ep_sb[:], bias_ep_d[:])
            w_joint_sb = cp.tile([128, JC, V], BF16)
            bias_j_sb = cp.tile([128, V], BF16)
            wj_re = w_joint_d.rearrange("(c p) v -> p c v", p=128)
            nc.sync.dma_start(w_joint_sb[:, :, 0:512], wj_re[:, :, 0:512])
            nc.sync.dma_start(bias_j_sb[:], bias_j_d[:])
            nc.sync.dma_start(w_joint_sb[:, :, 512:1024], wj_re[:, :, 512:1024])
            # ---- PE warm-up: ~2us of discarded matmuls so the clock
            # ramp (HAM) is at full rate when the projections start ----
            wtile = cp.tile([128, 512], BF16)
            nc.gpsimd.memset(wtile[:], 0.0)
            wpsum = ps_mm.tile([128, 512], F32, tag="mm")
            for i in range(10):
                nc.tensor.matmul(wpsum[:], wtile[:, 0:128], wtile[:],
                                 start=(i == 0), stop=(i == 9))
            wdump = cp.tile([128, 16], F32)
            nc.scalar.copy(wdump[:], wpsum[:, 0:16])

            # ---- projections (transposed layout [j, row]), interleaved
            # per-jc so the first slab chunk is ready ASAP ----
            encP = cp.tile([128, JC, BT], BF16)
            predP = cp.tile([128, JC, B * U], BF16)
            slab0 = slab_pool.tile([128, JC, 512], BF16)

            def slab_add_tanh(slab, b, tg, jc):
                bt0 = b * TL + tg * 8
                # [128, 8t, 64u] = pred[., u] (bcast t) + enc[., t] (bcast u)
                p_ap = predP[:, jc, b * U:(b + 1) * U]
                in0 = AP(p_ap.tensor, p_ap.offset,
                         [p_ap.ap[0], [0, 8], [1, U]])
                e_ap = encP[:, jc, bt0:bt0 + 8]
                in1 = AP(e_ap.tensor, e_ap.offset,
                         [e_ap.ap[0], [1, 8], [0, U]])
                dst = slab[:, jc, :].rearrange("p (t u) -> p t u", t=8)
                nc.vector.tensor_tensor(dst, in0, in1, ADD)
                nc.scalar.activation(slab[:, jc, :], slab[:, jc, :], TANH)

            for jc in range(JC):
                pe = ps_mm.tile([128, 512], F32, tag="mm")
                for ec in range(JC):
                    nc.tensor.matmul(pe[:, 0:BT],
                                     w_enc_sb[:, ec, jc * 128:(jc + 1) * 128],
                                     encT[:, ec, :],
                                     start=(ec == 0), stop=(ec == JC - 1))
                nc.scalar.copy(encP[:, jc, :], pe[:, 0:BT])
                pp = ps_mm.tile([128, 512], F32, tag="mm")
                for ec in range(JC):
                    nc.tensor.matmul(pp[:, 0:256],
                                     w_pred_sb[:, ec, jc * 128:(jc + 1) * 128],
                                     predT[:, ec, :],
                                     start=(ec == 0), stop=(ec == JC - 1))
                # fused (b_enc + b_pred) bias add during PSUM evacuation
                nc.vector.tensor_scalar_add(predP[:, jc, :], pp[:, 0:256],
                                            bias_ep_sb[:, jc:jc + 1])
                # first slab chunk rides right behind its projections
                slab_add_tanh(slab0, 0, 0, jc)

            # ---- main loop: per (b, t-group of 8) slab -> 4 out tiles.
            # Slabs are produced one group ahead of their tiles so DVE/ACT
            # slab work never gates the PE matmul stream. ----
            slabs = {(0, 0): slab0}

            def make_slab(b, tg):
                s = slab_pool.tile([128, JC, 512], BF16, name="slab",
                                   tag="slab")
                for jc in range(JC):
                    slab_add_tanh(s, b, tg, jc)
                slabs[(b, tg)] = s

            groups = [(b, tg) for b in range(B) for tg in range(TG)]
            for gi, (b, tg) in enumerate(groups):
                    if gi + 1 < len(groups):
                        make_slab(*groups[gi + 1])
                    slab = slabs.pop((b, tg))

                    ots = {}

                    def half_tile(pi, vh):
                        tidx = (b * TG + tg) * 4 + pi
                        if pi not in ots:
                            ots[pi] = out_pool.tile([128, V], F32,
                                                    name="ot", tag="ot")
                        ot = ots[pi]
                        vs = slice(vh * 512, (vh + 1) * 512)
                        po = ps_mm.tile([128, 512], F32, tag="mm")
                        for jc in range(JC):
                            nc.tensor.matmul(
                                po[:],
                                slab[:, jc, pi * 128:(pi + 1) * 128],
                                w_joint_sb[:, jc, vs],
                                start=(jc == 0), stop=(jc == JC - 1))
                        if tidx % 2 == 0 or tidx >= NT - 8:
                            # DVE: fused PSUM evac + b_joint add
                            nc.vector.tensor_tensor(
                                ot[:, vs], po[:], bias_j_sb[:, vs], ADD)
                        else:
                            # ACT evacuates, GPSIMD adds bias in place
                            nc.scalar.copy(ot[:, vs], po[:])
                            nc.gpsimd.tensor_add(
                                ot[:, vs], ot[:, vs], bias_j_sb[:, vs])

                    if gi < 2:
                        # vh0 for all 4 tiles first: hides the w_joint vh1
                        # DMA arrival behind the first 3.4us of matmuls
                        order = [(pi, 0) for pi in range(4)] +                                 [(pi, 1) for pi in range(4)]
                    else:
                        order = [(pi, vh) for pi in range(4) for vh in range(2)]
                    for pi, vh in order:
                        half_tile(pi, vh)
                        tidx = (b * TG + tg) * 4 + pi
                        vs = slice(vh * 512, (vh + 1) * 512)
                        nc.sync.dma_start(
                            out_d[tidx * 128:(tidx + 1) * 128, vs],
                            ots[pi][:, vs])
    nc.compile()
    return nc


def _get_nc():
    if "nc" not in _cache:
        _cache["nc"] = _build()
    return _cache["nc"]


def make_in_maps(enc_out, pred_out, W_enc, b_enc, W_pred, b_pred, W_joint, b_joint):
    predT = np.ascontiguousarray(
        np.asarray(pred_out, dtype=np.float32).reshape(B * U, JC, 128)
        .transpose(2, 1, 0).reshape(128, JC * B * U)).astype(NP_BF16)
    bias_ep = np.ascontiguousarray(
        (np.asarray(b_enc, dtype=np.float32)
         + np.asarray(b_pred, dtype=np.float32)).reshape(JC, 128).T)
    bias_j = np.ascontiguousarray(
        np.broadcast_to(np.asarray(b_joint, dtype=np.float32), (128, V))
    ).astype(NP_BF16)
    w_enc = np.ascontiguousarray(W_enc, dtype=np.float32).astype(NP_BF16)
    w_pred = np.ascontiguousarray(W_pred, dtype=np.float32).astype(NP_BF16)
    w_joint = np.ascontiguousarray(W_joint, dtype=np.float32).astype(NP_BF16)
    enc_f32 = np.asarray(enc_out, dtype=np.float32)
    in_maps = []
    for i in range(N_CORES):
        enc_slab = np.ascontiguousarray(
            enc_f32[:, i * TL:(i + 1) * TL, :].reshape(BT, JC, 128)
            .transpose(2, 1, 0).reshape(128, JC * BT)).astype(NP_BF16)
        in_maps.append({
            "enc": enc_slab, "pred": predT,
            "w_enc": w_enc, "w_pred": w_pred, "w_joint": w_joint,
            "bias_ep": bias_ep, "bias_j": bias_j,
        })
    return in_maps


def assemble(results):
    return np.concatenate(
        [r["out"].reshape(B, TL, U, V) for r in results], axis=1)


def _axon_active():
    try:
        from concourse.bass_utils import axon_active
        return axon_active()
    except Exception:
        return False


def _get_fast_runner(nc):
    """Cached jit dispatch (axon path). Same mechanism as
    bass2jax.run_bass_via_pjrt, built once so repeat kernel() calls skip
    the per-call trace/lower/compile."""
    if "runner" in _cache:
        return _cache["runner"]

    import jax
    from jax.sharding import Mesh, PartitionSpec, NamedSharding
    from jax.experimental.shard_map import shard_map
    from concourse.bass2jax import (
        _bass_exec_p, install_neuronx_cc_hook, partition_id_tensor)

    install_neuronx_cc_hook()
    partition_name = nc.partition_id_tensor.name if nc.partition_id_tensor else None
    in_names, out_names, out_avals, zero_outs = [], [], [], []
    for alloc in nc.m.functions[0].allocations:
        if not isinstance(alloc, mybir.MemoryLocationSet):
            continue
        name = alloc.memorylocations[0].name
        if alloc.kind == "ExternalInput":
            if name != partition_name:
                in_names.append(name)
        elif alloc.kind == "ExternalOutput":
            shape = tuple(alloc.tensor_shape)
            dtype = mybir.dt.np(alloc.dtype)
            out_names.append(name)
            out_avals.append(jax.core.ShapedArray(shape, dtype))
            zero_outs.append(np.zeros(shape, dtype))
    n_params = len(in_names)
    n_outs = len(out_avals)
    all_names = in_names + out_names
    if partition_name is not None:
        all_names = all_names + [partition_name]

    def _body(*args):
        operands = list(args)
        if partition_name is not None:
            operands.append(partition_id_tensor())
        outs = _bass_exec_p.bind(
            *operands, out_avals=tuple(out_avals), in_names=tuple(all_names),
            out_names=tuple(out_names), lowering_input_output_aliases=(),
            sim_require_finite=True, sim_require_nnan=True, nc=nc)
        return tuple(outs)

    devices = jax.devices()[:N_CORES]
    mesh = Mesh(np.asarray(devices), ("core",))
    sharded = jax.jit(
        shard_map(_body, mesh=mesh,
                  in_specs=(PartitionSpec("core"),) * (n_params + n_outs),
                  out_specs=(PartitionSpec("core"),) * n_outs,
                  check_rep=False),
        keep_unused=True)
    sh = NamedSharding(mesh, PartitionSpec("core"))
    zeros_dev = [
        jax.device_put(np.zeros((N_CORES * z.shape[0], *z.shape[1:]), z.dtype), sh)
        for z in zero_outs]

    def run(in_maps):
        concat_in = [
            jax.device_put(
                np.concatenate([in_maps[c][n] for c in range(N_CORES)], axis=0), sh)
            for n in in_names]
        outs = sharded(*concat_in, *zeros_dev)
        res = []
        for c in range(N_CORES):
            m = {}
            for i, n in enumerate(out_names):
                rows = out_avals[i].shape[0]
                m[n] = np.asarray(outs[i][c * rows:(c + 1) * rows])
            res.append(m)
        return res

    _cache["runner"] = run
    return run


def kernel(enc_out, pred_out, W_enc, b_enc, W_pred, b_pred, W_joint, b_joint):
    nc = _get_nc()
    in_maps = make_in_maps(enc_out, pred_out, W_enc, b_enc, W_pred, b_pred,
                           W_joint, b_joint)
    if _axon_active():
        results = _get_fast_runner(nc)(in_maps)
    else:
        results = bass_utils.run_bass_kernel_spmd(
            nc, in_maps, list(range(N_CORES))).results
    return assemble(results)
